# revision 15
# baseline (speedup 1.0000x reference)
"""Trainium2 Bass kernel for nn_Block_85633057947963 (dense transformer block).

Strategy:
  - Data-parallel over batch: 16 batches -> 8 cores x 2 batches (2048 tokens/core).
  - BatchNorm (fixed running stats) folded into each linear on the host:
        stage(x) = relu(x @ w_eff + b_eff),
        w_eff = w.T * s, b_eff = (b - m) * s + be, s = g / sqrt(v + eps).
  - Linear attention reassociated: (q k^T * 0.125) v == q (0.125 * k^T v),
    turning two [N,N] matmuls into two tiny [64,64]-contraction matmuls.
  - Activations kept in transposed layout [C, tokens] (channels on SBUF
    partitions) so every linear is matmul(lhsT=w_eff, rhs=actT) with zero
    on-device transposes, and per-channel bias+ReLU ride the ACT engine's
    free affine. k and v are additionally produced in natural layout
    [tokens, C] for the k^T v contraction.
  - Matmul operands in bf16 (1 cyc/row on the PE vs 2 for fp32), fp32 PSUM
    accumulation, fp32 epilogues; the x1 residual is kept in fp32.
"""

import numpy as np
import ml_dtypes

import concourse.bass as bass
import concourse.tile as tile
from concourse import bacc, mybir
from concourse.bass_utils import run_bass_kernel_spmd

# Problem shapes (hardcoded per spec)
B, N, C, H, HID = 16, 1024, 512, 8, 2048
BN_EPS = 1e-5
NCORES = 8
BPC = B // NCORES          # batches per core = 2
NT = BPC * N               # tokens per core = 2048
P = 128
KC = C // P                # 4 chunks of input channels
HC = HID // P              # 16 chunks of hidden channels
DH = C // H                # 64 = head dim
TCH = 512                  # token chunk (matmul free dim)
NTC = NT // TCH            # 4 token chunks per core
TPB = N // P               # 8 chunks of 128 tokens per batch

F32 = mybir.dt.float32
BF16 = mybir.dt.bfloat16
NPBF = ml_dtypes.bfloat16
RELU = mybir.ActivationFunctionType.Relu
ADD = mybir.AluOpType.add


def _build_nc(debug=False):
    nc = bacc.Bacc("TRN2", target_bir_lowering=False, debug=debug,
                   num_devices=NCORES)

    x_d = nc.dram_tensor("x_d", [P, KC, NT], BF16, kind="ExternalInput").ap()
    w_in = {}
    for nm, kc, cout in (("q", KC, C), ("k", KC, C), ("v", KC, C),
                         ("p", KC, C), ("f1", KC, HID), ("f2", HC, C)):
        w_in[nm] = nc.dram_tensor(f"w_{nm}", [P, kc, cout], BF16,
                                  kind="ExternalInput").ap()
    b_in = {}
    for nm, nch in (("q", KC), ("p", KC), ("f1", HC), ("f2", KC)):
        b_in[nm] = nc.dram_tensor(f"b_{nm}", [P, nch], F32,
                                  kind="ExternalInput").ap()
    for nm in ("k", "v"):
        b_in[nm] = nc.dram_tensor(f"b_{nm}", [P, C], F32,
                                  kind="ExternalInput").ap()
    out_d = nc.dram_tensor("out", [P, KC, NT], F32, kind="ExternalOutput").ap()

    with tile.TileContext(nc) as tc:
        with (
            tc.tile_pool(name="wpool", bufs=1) as wpool,
            tc.tile_pool(name="bpool", bufs=1) as bpool,
            tc.tile_pool(name="actD", bufs=2) as actD,       # x / qT / oT
            tc.tile_pool(name="kv", bufs=2) as kvpool,       # k_nat, v_nat
            tc.tile_pool(name="x1f", bufs=1) as x1pool,      # x1 fp32
            tc.tile_pool(name="hp", bufs=1) as hpool,        # FFN hidden
            tc.tile_pool(name="tmpf", bufs=4) as tmpf,       # fp32 [P,TCH]
            tc.tile_pool(name="tmpd", bufs=3) as tmpd,       # bf16 [P,TCH]
            tc.tile_pool(name="castd", bufs=5) as castd,     # x1 bf16 slices
            tc.tile_pool(name="xres", bufs=3) as xres,       # x stream for res1
            tc.tile_pool(name="spool", bufs=9) as spool,     # S head-pair tiles
            tc.tile_pool(name="psA", bufs=7, space="PSUM") as psA,
        ):
            # ---- resident weights & biases.  DMA order matters: the input
            # stream is ~8MB at per-core HBM bandwidth (~22us serial), so
            # emit (biases, w_q, x-chunks) first and the FFN weights last,
            # letting the first q matmuls start after ~1MB has landed.
            wt = {}
            bt = {}

            def load_w(nm, kc, cout, eng):
                t = wpool.tile([P, kc, cout], BF16, tag=f"w_{nm}")
                eng.dma_start(t[:], w_in[nm])
                wt[nm] = t

            # Early-needed small weights + biases issue from the (otherwise
            # idle) GpSimd queue, in parallel with Sync issuing the x chunks.
            # The big FFN weights stay on Sync BEHIND the x chunks so their
            # transfers don't steal HBM bandwidth from the critical prefix.
            load_w("q", KC, C, nc.gpsimd)
            xT = actD.tile([P, KC, NT], BF16, tag="big")
            for t in range(NTC):
                nc.sync.dma_start(xT[:, :, t * TCH:(t + 1) * TCH],
                                  x_d[:, :, t * TCH:(t + 1) * TCH])
            load_w("k", KC, C, nc.gpsimd)
            load_w("v", KC, C, nc.gpsimd)
            for nm, nch in (("q", KC), ("p", KC), ("f1", HC), ("f2", KC)):
                t = bpool.tile([P, nch], F32, tag=f"b_{nm}")
                nc.gpsimd.dma_start(t[:], b_in[nm])
                bt[nm] = t
            for nm in ("k", "v"):
                t = bpool.tile([P, C], F32, tag=f"b_{nm}")
                nc.gpsimd.dma_start(t[:], b_in[nm])
                bt[nm] = t
            load_w("p", KC, C, nc.sync)
            load_w("f1", KC, HID, nc.sync)
            load_w("f2", HC, C, nc.sync)

            # explicit zero bias (avoids a const-tensor preamble load)
            zbias = bpool.tile([P, 1], F32, tag="zb")
            nc.vector.memset(zbias[:], 0.0)

            # pre-warm the PE HAM clock gate with junk matmuls while the
            # input DMAs stream in (results discarded)
            warm_w = bpool.tile([P, TCH], BF16, tag="warm")
            nc.vector.memset(warm_w[:], 0.0)
            ps_warm = psA.tile([P, TCH], F32, tag="warm", bufs=1)
            for _ in range(11):
                nc.tensor.matmul(ps_warm[:], warm_w[:, :P], warm_w[:],
                                 start=True, stop=True)

            # ---- phase 1: projections
            qT = actD.tile([P, KC, NT], BF16, tag="big")
            kN = kvpool.tile([P, BPC * TPB, C], BF16, tag="kv")
            vN = kvpool.tile([P, BPC * TPB, C], BF16, tag="kv")

            for t in range(NTC):          # q: transposed output; token-outer
                for ch in range(KC):      # so MMs start once x chunk 0 lands
                    ps = psA.tile([P, TCH], F32, tag="mm")
                    for ks in range(KC):
                        nc.tensor.matmul(ps[:], wt["q"][:, ks, ch * P:(ch + 1) * P],
                                         xT[:, ks, t * TCH:(t + 1) * TCH],
                                         start=(ks == 0), stop=(ks == KC - 1))
                    nc.scalar.activation(qT[:, ch, t * TCH:(t + 1) * TCH], ps[:],
                                         RELU, bias=bt["q"][:, ch:ch + 1])

            for nm, dst in (("k", kN), ("v", vN)):   # k, v: natural output
                for tch in range(BPC * TPB):
                    ps = psA.tile([P, C], F32, tag="mm")
                    for ks in range(KC):
                        nc.tensor.matmul(ps[:], xT[:, ks, tch * P:(tch + 1) * P],
                                         wt[nm][:, ks, :],
                                         start=(ks == 0), stop=(ks == KC - 1))
                    tmp = tmpd.tile([P, C], BF16, tag="kvtmp")
                    nc.vector.tensor_tensor(tmp[:], ps[:], bt[nm][:], ADD)
                    nc.scalar.activation(dst[:, tch, :], tmp[:], RELU, bias=zbias[:])

            # ---- phase 2: attention (associative).  The two heads of each
            # 128-channel pair are packed into one PSUM tile at partition
            # bases 0/64 (tile_position auto-derives from the AP bases), so
            # the two M=64 matmul streams run on disjoint PE column groups
            # and each epilogue is a single [128, .] ACT op.
            oT = actD.tile([P, KC, NT], BF16, tag="big")
            Sps = {}
            for b in range(BPC):           # pass 1: all k^T v chains
                for hp in range(KC):
                    Sp = spool.tile([P, DH], BF16, tag="S")
                    ps_full = psA.tile([P, TCH], F32, tag="mm", name="ps_s")
                    ps = ps_full[:, :DH]
                    for j in range(TPB):
                        tch = b * TPB + j
                        for sub in range(2):
                            h = hp * 2 + sub
                            nc.tensor.matmul(ps[sub * DH:(sub + 1) * DH, :],
                                             kN[:, tch, h * DH:(h + 1) * DH],
                                             vN[:, tch, h * DH:(h + 1) * DH],
                                             start=(j == 0), stop=(j == TPB - 1),
                                             skip_group_check=True)
                    nc.scalar.mul(Sp[:], ps[:], 0.125)
                    Sps[b, hp] = Sp
            # pass 2 (o = q S) interleaved per token chunk with phase 3
            # (p projection + residual 1) so the PE never waits on epilogues
            x1 = x1pool.tile([P, KC, NT], F32, tag="x1")
            for tg in range(NTC):          # global token chunk
                b, t = divmod(tg, N // TCH)
                tok0 = b * N + t * TCH
                for hp in range(KC):
                    Sp = Sps[b, hp]
                    ps_o = psA.tile([P, TCH], F32, tag="mm")
                    for sub in range(2):
                        nc.tensor.matmul(ps_o[sub * DH:(sub + 1) * DH, :],
                                         Sp[sub * DH:(sub + 1) * DH, :],
                                         qT[sub * DH:(sub + 1) * DH, hp,
                                            tok0:tok0 + TCH],
                                         start=True, stop=True,
                                         skip_group_check=True)
                    nc.scalar.activation(oT[:, hp, tok0:tok0 + TCH],
                                         ps_o[:], RELU, bias=zbias[:])
                if tg == 0:
                    continue  # p for chunk tg-1 emitted below once o ready
                for ch in range(KC):
                    tp = tg - 1
                    ps = psA.tile([P, TCH], F32, tag="mm")
                    for ks in range(KC):
                        nc.tensor.matmul(ps[:], wt["p"][:, ks, ch * P:(ch + 1) * P],
                                         oT[:, ks, tp * TCH:(tp + 1) * TCH],
                                         start=(ks == 0), stop=(ks == KC - 1))
                    tf = tmpf.tile([P, TCH], F32, tag="tf")
                    nc.scalar.activation(tf[:], ps[:], RELU, bias=bt["p"][:, ch:ch + 1])
                    xr = xres.tile([P, TCH], BF16, tag="xr")
                    nc.sync.dma_start(xr[:], x_d[:, ch, tp * TCH:(tp + 1) * TCH])
                    nc.vector.tensor_tensor(x1[:, ch, tp * TCH:(tp + 1) * TCH],
                                            tf[:], xr[:], ADD)
            for ch in range(KC):           # p for the last token chunk
                tp = NTC - 1
                ps = psA.tile([P, TCH], F32, tag="mm")
                for ks in range(KC):
                    nc.tensor.matmul(ps[:], wt["p"][:, ks, ch * P:(ch + 1) * P],
                                     oT[:, ks, tp * TCH:(tp + 1) * TCH],
                                     start=(ks == 0), stop=(ks == KC - 1))
                tf = tmpf.tile([P, TCH], F32, tag="tf")
                nc.scalar.activation(tf[:], ps[:], RELU, bias=bt["p"][:, ch:ch + 1])
                xr = xres.tile([P, TCH], BF16, tag="xr")
                nc.sync.dma_start(xr[:], x_d[:, ch, tp * TCH:(tp + 1) * TCH])
                nc.vector.tensor_tensor(x1[:, ch, tp * TCH:(tp + 1) * TCH],
                                        tf[:], xr[:], ADD)

            # ---- phase 4: FFN + residual 2, per token chunk
            for t in range(NTC):
                x1d = []
                for ks in range(KC):
                    cd = castd.tile([P, TCH], BF16, tag="x1d")
                    nc.vector.tensor_copy(cd[:], x1[:, ks, t * TCH:(t + 1) * TCH])
                    x1d.append(cd)
                hT = hpool.tile([P, HC, TCH], BF16, tag="h")
                for hch in range(HC):
                    ps = psA.tile([P, TCH], F32, tag="mm")
                    for ks in range(KC):
                        nc.tensor.matmul(ps[:], wt["f1"][:, ks, hch * P:(hch + 1) * P],
                                         x1d[ks][:],
                                         start=(ks == 0), stop=(ks == KC - 1))
                    # relu(psum + bias): alternate DVE/ACT to balance engines
                    if hch % 2 == 0:
                        nc.vector.tensor_scalar(hT[:, hch, :], ps[:],
                                                bt["f1"][:, hch:hch + 1], 0.0,
                                                ADD, mybir.AluOpType.max)
                    else:
                        nc.scalar.activation(hT[:, hch, :], ps[:], RELU,
                                             bias=bt["f1"][:, hch:hch + 1])
                for ch in range(KC):
                    ps = psA.tile([P, TCH], F32, tag="mm")
                    for ks in range(HC):
                        nc.tensor.matmul(ps[:], wt["f2"][:, ks, ch * P:(ch + 1) * P],
                                         hT[:, ks, :],
                                         start=(ks == 0), stop=(ks == HC - 1))
                    tf = tmpf.tile([P, TCH], F32, tag="tf")
                    nc.scalar.activation(tf[:], ps[:], RELU, bias=bt["f2"][:, ch:ch + 1])
                    of = tmpf.tile([P, TCH], F32, tag="tf")
                    nc.vector.tensor_tensor(of[:], tf[:],
                                            x1[:, ch, t * TCH:(t + 1) * TCH], ADD)
                    nc.sync.dma_start(out_d[:, ch, t * TCH:(t + 1) * TCH], of[:])

    nc.compile()
    return nc


def _eff_params(inputs, pref):
    w = inputs[pref + "_w"].astype(np.float32)
    b = inputs[pref + "_b"].astype(np.float32)
    g = inputs[pref + "_g"].astype(np.float32)
    be = inputs[pref + "_be"].astype(np.float32)
    m = inputs[pref + "_m"].astype(np.float32)
    v = inputs[pref + "_v"].astype(np.float32)
    s = g / np.sqrt(v + BN_EPS)
    w_eff = (w.T * s).astype(np.float32)          # [C_in, C_out]
    b_eff = ((b - m) * s + be).astype(np.float32)
    return w_eff, b_eff


def _wlayout(w_eff):
    """[C_in, C_out] -> [P, C_in//P, C_out] with channel-in striped on partitions."""
    cin, cout = w_eff.shape
    return np.ascontiguousarray(
        w_eff.reshape(cin // P, P, cout).transpose(1, 0, 2)).astype(NPBF)


def _blayout_T(b_eff):
    """per-channel bias -> [P, nch] (channel chunks on free dim)."""
    n = b_eff.shape[0]
    return np.ascontiguousarray(b_eff.reshape(n // P, P).T).astype(np.float32)


_CACHE = {}


def _get_nc():
    if "nc" not in _CACHE:
        _CACHE["nc"] = _build_nc(debug=False)
    return _CACHE["nc"]


def _common_inputs(inputs):
    common = {}
    for nm in ("q", "k", "v", "p", "f1", "f2"):
        w_eff, b_eff = _eff_params(inputs, nm)
        common[f"w_{nm}"] = _wlayout(w_eff)
        if nm in ("k", "v"):
            common[f"b_{nm}"] = np.ascontiguousarray(
                np.broadcast_to(b_eff[None, :], (P, C))).astype(np.float32)
        else:
            common[f"b_{nm}"] = _blayout_T(b_eff)
    return common


def _shard_x(x, i):
    """core i's x shard -> [P, KC, NT] bf16 transposed layout."""
    xc = np.asarray(x[i * BPC:(i + 1) * BPC], dtype=np.float32)  # (BPC, N, C)
    xt = xc.reshape(NT, C).T                                     # [C, NT]
    xt = xt.reshape(KC, P, NT).transpose(1, 0, 2)                # [P, KC, NT]
    return np.ascontiguousarray(xt).astype(NPBF)


def _unshard_out(res):
    """[P, KC, NT] f32 -> (BPC, N, C) f32."""
    yt = res.transpose(1, 0, 2).reshape(C, NT)                   # [C, NT]
    return np.ascontiguousarray(yt.T.reshape(BPC, N, C))


def run(inputs, trace=False, **kwargs):
    nc = _get_nc()
    common = _common_inputs(inputs)
    in_maps = [dict(common, x_d=_shard_x(inputs["x"], i)) for i in range(NCORES)]
    res = run_bass_kernel_spmd(nc, in_maps, core_ids=list(range(NCORES)),
                               trace=trace, **kwargs)
    y = np.empty((B, N, C), dtype=np.float32)
    for i in range(NCORES):
        y[i * BPC:(i + 1) * BPC] = _unshard_out(res.results[i]["out"])
    return y, res


def kernel(**inputs):
    y, _ = run(inputs, trace=False)
    return y


# revision 16
# speedup vs baseline: 1.0136x; 1.0136x over previous
"""Trainium2 Bass kernel for nn_Block_85633057947963 (dense transformer block).

Strategy:
  - Data-parallel over batch: 16 batches -> 8 cores x 2 batches (2048 tokens/core).
  - BatchNorm (fixed running stats) folded into each linear on the host:
        stage(x) = relu(x @ w_eff + b_eff),
        w_eff = w.T * s, b_eff = (b - m) * s + be, s = g / sqrt(v + eps).
  - Linear attention reassociated: (q k^T * 0.125) v == q (0.125 * k^T v),
    turning two [N,N] matmuls into two tiny [64,64]-contraction matmuls.
  - Activations kept in transposed layout [C, tokens] (channels on SBUF
    partitions) so every linear is matmul(lhsT=w_eff, rhs=actT) with zero
    on-device transposes, and per-channel bias+ReLU ride the ACT engine's
    free affine. k and v are additionally produced in natural layout
    [tokens, C] for the k^T v contraction.
  - Matmul operands in bf16 (1 cyc/row on the PE vs 2 for fp32), fp32 PSUM
    accumulation, fp32 epilogues; the x1 residual is kept in fp32.
"""

import numpy as np
import ml_dtypes

import concourse.bass as bass
import concourse.tile as tile
from concourse import bacc, mybir
from concourse.bass_utils import run_bass_kernel_spmd

# Problem shapes (hardcoded per spec)
B, N, C, H, HID = 16, 1024, 512, 8, 2048
BN_EPS = 1e-5
NCORES = 8
BPC = B // NCORES          # batches per core = 2
NT = BPC * N               # tokens per core = 2048
P = 128
KC = C // P                # 4 chunks of input channels
HC = HID // P              # 16 chunks of hidden channels
DH = C // H                # 64 = head dim
TCH = 512                  # token chunk (matmul free dim)
NTC = NT // TCH            # 4 token chunks per core
TPB = N // P               # 8 chunks of 128 tokens per batch

F32 = mybir.dt.float32
BF16 = mybir.dt.bfloat16
NPBF = ml_dtypes.bfloat16
RELU = mybir.ActivationFunctionType.Relu
ADD = mybir.AluOpType.add


def _build_nc(debug=False):
    nc = bacc.Bacc("TRN2", target_bir_lowering=False, debug=debug,
                   num_devices=NCORES)

    x_d = nc.dram_tensor("x_d", [P, KC, NT], BF16, kind="ExternalInput").ap()
    w_in = {}
    for nm, kc, cout in (("q", KC, C), ("k", KC, C), ("v", KC, C),
                         ("p", KC, C), ("f1", KC, HID), ("f2", HC, C)):
        w_in[nm] = nc.dram_tensor(f"w_{nm}", [P, kc, cout], BF16,
                                  kind="ExternalInput").ap()
    b_in = {}
    for nm, nch in (("q", KC), ("p", KC), ("f1", HC), ("f2", KC)):
        b_in[nm] = nc.dram_tensor(f"b_{nm}", [P, nch], F32,
                                  kind="ExternalInput").ap()
    for nm in ("k", "v"):
        b_in[nm] = nc.dram_tensor(f"b_{nm}", [P, C], F32,
                                  kind="ExternalInput").ap()
    out_d = nc.dram_tensor("out", [P, KC, NT], F32, kind="ExternalOutput").ap()

    with tile.TileContext(nc) as tc:
        with (
            tc.tile_pool(name="wpool", bufs=1) as wpool,
            tc.tile_pool(name="bpool", bufs=1) as bpool,
            tc.tile_pool(name="actD", bufs=2) as actD,       # x / qT / oT
            tc.tile_pool(name="kv", bufs=2) as kvpool,       # k_nat, v_nat
            tc.tile_pool(name="x1f", bufs=1) as x1pool,      # x1 fp32
            tc.tile_pool(name="hp", bufs=1) as hpool,        # FFN hidden
            tc.tile_pool(name="tmpf", bufs=4) as tmpf,       # fp32 [P,TCH]
            tc.tile_pool(name="tmpd", bufs=3) as tmpd,       # bf16 [P,TCH]
            tc.tile_pool(name="castd", bufs=5) as castd,     # x1 bf16 slices
            tc.tile_pool(name="xres", bufs=3) as xres,       # x stream for res1
            tc.tile_pool(name="spool", bufs=9) as spool,     # S head-pair tiles
            tc.tile_pool(name="psA", bufs=7, space="PSUM") as psA,
        ):
            # ---- resident weights & biases.  DMA order matters: the input
            # stream is ~8MB at per-core HBM bandwidth (~22us serial), so
            # emit (biases, w_q, x-chunks) first and the FFN weights last,
            # letting the first q matmuls start after ~1MB has landed.
            wt = {}
            bt = {}

            def load_w(nm, kc, cout, eng):
                t = wpool.tile([P, kc, cout], BF16, tag=f"w_{nm}")
                eng.dma_start(t[:], w_in[nm])
                wt[nm] = t

            # Early-needed small weights + biases issue from the (otherwise
            # idle) GpSimd queue, in parallel with Sync issuing the x chunks.
            # The big FFN weights stay on Sync BEHIND the x chunks so their
            # transfers don't steal HBM bandwidth from the critical prefix.
            load_w("q", KC, C, nc.gpsimd)
            xT = actD.tile([P, KC, NT], BF16, tag="big")
            for t in range(NTC):
                eng = nc.sync if t < 2 else nc.scalar
                eng.dma_start(xT[:, :, t * TCH:(t + 1) * TCH],
                              x_d[:, :, t * TCH:(t + 1) * TCH])
            load_w("k", KC, C, nc.gpsimd)
            load_w("v", KC, C, nc.gpsimd)
            for nm, nch in (("q", KC), ("p", KC), ("f1", HC), ("f2", KC)):
                t = bpool.tile([P, nch], F32, tag=f"b_{nm}")
                nc.gpsimd.dma_start(t[:], b_in[nm])
                bt[nm] = t
            for nm in ("k", "v"):
                t = bpool.tile([P, C], F32, tag=f"b_{nm}")
                nc.gpsimd.dma_start(t[:], b_in[nm])
                bt[nm] = t
            load_w("p", KC, C, nc.sync)
            load_w("f1", KC, HID, nc.sync)
            load_w("f2", HC, C, nc.sync)

            # explicit zero bias (avoids a const-tensor preamble load)
            zbias = bpool.tile([P, 1], F32, tag="zb")
            nc.vector.memset(zbias[:], 0.0)

            # pre-warm the PE HAM clock gate with junk matmuls while the
            # input DMAs stream in (results discarded)
            warm_w = bpool.tile([P, TCH], BF16, tag="warm")
            nc.vector.memset(warm_w[:], 0.0)
            ps_warm = psA.tile([P, TCH], F32, tag="warm", bufs=1)
            for _ in range(11):
                nc.tensor.matmul(ps_warm[:], warm_w[:, :P], warm_w[:],
                                 start=True, stop=True)

            # ---- phase 1: projections
            qT = actD.tile([P, KC, NT], BF16, tag="big")
            kN = kvpool.tile([P, BPC * TPB, C], BF16, tag="kv")
            vN = kvpool.tile([P, BPC * TPB, C], BF16, tag="kv")

            for t in range(NTC):          # q: transposed output; token-outer
                for ch in range(KC):      # so MMs start once x chunk 0 lands
                    ps = psA.tile([P, TCH], F32, tag="mm")
                    for ks in range(KC):
                        nc.tensor.matmul(ps[:], wt["q"][:, ks, ch * P:(ch + 1) * P],
                                         xT[:, ks, t * TCH:(t + 1) * TCH],
                                         start=(ks == 0), stop=(ks == KC - 1))
                    nc.scalar.activation(qT[:, ch, t * TCH:(t + 1) * TCH], ps[:],
                                         RELU, bias=bt["q"][:, ch:ch + 1])

            for nm, dst in (("k", kN), ("v", vN)):   # k, v: natural output
                for tch in range(BPC * TPB):
                    ps = psA.tile([P, C], F32, tag="mm")
                    for ks in range(KC):
                        nc.tensor.matmul(ps[:], xT[:, ks, tch * P:(tch + 1) * P],
                                         wt[nm][:, ks, :],
                                         start=(ks == 0), stop=(ks == KC - 1))
                    tmp = tmpd.tile([P, C], BF16, tag="kvtmp")
                    nc.vector.tensor_tensor(tmp[:], ps[:], bt[nm][:], ADD)
                    nc.scalar.activation(dst[:, tch, :], tmp[:], RELU, bias=zbias[:])

            # ---- phase 2: attention (associative).  The two heads of each
            # 128-channel pair are packed into one PSUM tile at partition
            # bases 0/64 (tile_position auto-derives from the AP bases), so
            # the two M=64 matmul streams run on disjoint PE column groups
            # and each epilogue is a single [128, .] ACT op.
            oT = actD.tile([P, KC, NT], BF16, tag="big")
            Sps = {}
            for b in range(BPC):           # pass 1: all k^T v chains
                for hp0 in (0, 2):         # two chains interleaved
                    pss = {}
                    for hp in (hp0, hp0 + 1):
                        Sps[b, hp] = spool.tile([P, DH], BF16, tag="S",
                                                name=f"Sp_{b}_{hp}")
                        ps_full = psA.tile([P, TCH], F32, tag="mm",
                                           name=f"ps_s{b}_{hp}")
                        pss[hp] = ps_full[:, :DH]
                    for j in range(TPB):
                        tch = b * TPB + j
                        for hp in (hp0, hp0 + 1):
                            for sub in range(2):
                                h = hp * 2 + sub
                                nc.tensor.matmul(
                                    pss[hp][sub * DH:(sub + 1) * DH, :],
                                    kN[:, tch, h * DH:(h + 1) * DH],
                                    vN[:, tch, h * DH:(h + 1) * DH],
                                    start=(j == 0), stop=(j == TPB - 1),
                                    skip_group_check=True)
                    for hp in (hp0, hp0 + 1):
                        nc.scalar.mul(Sps[b, hp][:], pss[hp][:], 0.125)
            # pass 2 (o = q S) interleaved per token chunk with phase 3
            # (p projection + residual 1) so the PE never waits on epilogues
            x1 = x1pool.tile([P, KC, NT], F32, tag="x1")
            for tg in range(NTC):          # global token chunk
                b, t = divmod(tg, N // TCH)
                tok0 = b * N + t * TCH
                for hp in range(KC):
                    Sp = Sps[b, hp]
                    ps_o = psA.tile([P, TCH], F32, tag="mm")
                    for sub in range(2):
                        nc.tensor.matmul(ps_o[sub * DH:(sub + 1) * DH, :],
                                         Sp[sub * DH:(sub + 1) * DH, :],
                                         qT[sub * DH:(sub + 1) * DH, hp,
                                            tok0:tok0 + TCH],
                                         start=True, stop=True,
                                         skip_group_check=True)
                    nc.scalar.activation(oT[:, hp, tok0:tok0 + TCH],
                                         ps_o[:], RELU, bias=zbias[:])
                if tg == 0:
                    continue  # p for chunk tg-1 emitted below once o ready
                for ch in range(KC):
                    tp = tg - 1
                    ps = psA.tile([P, TCH], F32, tag="mm")
                    for ks in range(KC):
                        nc.tensor.matmul(ps[:], wt["p"][:, ks, ch * P:(ch + 1) * P],
                                         oT[:, ks, tp * TCH:(tp + 1) * TCH],
                                         start=(ks == 0), stop=(ks == KC - 1))
                    tf = tmpf.tile([P, TCH], F32, tag="tf")
                    nc.scalar.activation(tf[:], ps[:], RELU, bias=bt["p"][:, ch:ch + 1])
                    xr = xres.tile([P, TCH], BF16, tag="xr")
                    nc.sync.dma_start(xr[:], x_d[:, ch, tp * TCH:(tp + 1) * TCH])
                    nc.vector.tensor_tensor(x1[:, ch, tp * TCH:(tp + 1) * TCH],
                                            tf[:], xr[:], ADD)
            for ch in range(KC):           # p for the last token chunk
                tp = NTC - 1
                ps = psA.tile([P, TCH], F32, tag="mm")
                for ks in range(KC):
                    nc.tensor.matmul(ps[:], wt["p"][:, ks, ch * P:(ch + 1) * P],
                                     oT[:, ks, tp * TCH:(tp + 1) * TCH],
                                     start=(ks == 0), stop=(ks == KC - 1))
                tf = tmpf.tile([P, TCH], F32, tag="tf")
                nc.scalar.activation(tf[:], ps[:], RELU, bias=bt["p"][:, ch:ch + 1])
                xr = xres.tile([P, TCH], BF16, tag="xr")
                nc.sync.dma_start(xr[:], x_d[:, ch, tp * TCH:(tp + 1) * TCH])
                nc.vector.tensor_tensor(x1[:, ch, tp * TCH:(tp + 1) * TCH],
                                        tf[:], xr[:], ADD)

            # ---- phase 4: FFN + residual 2, per token chunk
            for t in range(NTC):
                x1d = []
                for ks in range(KC):
                    cd = castd.tile([P, TCH], BF16, tag="x1d")
                    nc.vector.tensor_copy(cd[:], x1[:, ks, t * TCH:(t + 1) * TCH])
                    x1d.append(cd)
                hT = hpool.tile([P, HC, TCH], BF16, tag="h")
                for hch in range(HC):
                    ps = psA.tile([P, TCH], F32, tag="mm")
                    for ks in range(KC):
                        nc.tensor.matmul(ps[:], wt["f1"][:, ks, hch * P:(hch + 1) * P],
                                         x1d[ks][:],
                                         start=(ks == 0), stop=(ks == KC - 1))
                    # relu(psum + bias): alternate DVE/ACT to balance engines
                    if hch % 2 == 0:
                        nc.vector.tensor_scalar(hT[:, hch, :], ps[:],
                                                bt["f1"][:, hch:hch + 1], 0.0,
                                                ADD, mybir.AluOpType.max)
                    else:
                        nc.scalar.activation(hT[:, hch, :], ps[:], RELU,
                                             bias=bt["f1"][:, hch:hch + 1])
                for ch in range(KC):
                    ps = psA.tile([P, TCH], F32, tag="mm")
                    for ks in range(HC):
                        nc.tensor.matmul(ps[:], wt["f2"][:, ks, ch * P:(ch + 1) * P],
                                         hT[:, ks, :],
                                         start=(ks == 0), stop=(ks == HC - 1))
                    tf = tmpf.tile([P, TCH], F32, tag="tf")
                    nc.scalar.activation(tf[:], ps[:], RELU, bias=bt["f2"][:, ch:ch + 1])
                    of = tmpf.tile([P, TCH], F32, tag="tf")
                    nc.vector.tensor_tensor(of[:], tf[:],
                                            x1[:, ch, t * TCH:(t + 1) * TCH], ADD)
                    nc.sync.dma_start(out_d[:, ch, t * TCH:(t + 1) * TCH], of[:])

    nc.compile()
    return nc


def _eff_params(inputs, pref):
    w = inputs[pref + "_w"].astype(np.float32)
    b = inputs[pref + "_b"].astype(np.float32)
    g = inputs[pref + "_g"].astype(np.float32)
    be = inputs[pref + "_be"].astype(np.float32)
    m = inputs[pref + "_m"].astype(np.float32)
    v = inputs[pref + "_v"].astype(np.float32)
    s = g / np.sqrt(v + BN_EPS)
    w_eff = (w.T * s).astype(np.float32)          # [C_in, C_out]
    b_eff = ((b - m) * s + be).astype(np.float32)
    return w_eff, b_eff


def _wlayout(w_eff):
    """[C_in, C_out] -> [P, C_in//P, C_out] with channel-in striped on partitions."""
    cin, cout = w_eff.shape
    return np.ascontiguousarray(
        w_eff.reshape(cin // P, P, cout).transpose(1, 0, 2)).astype(NPBF)


def _blayout_T(b_eff):
    """per-channel bias -> [P, nch] (channel chunks on free dim)."""
    n = b_eff.shape[0]
    return np.ascontiguousarray(b_eff.reshape(n // P, P).T).astype(np.float32)


_CACHE = {}


def _get_nc():
    if "nc" not in _CACHE:
        _CACHE["nc"] = _build_nc(debug=False)
    return _CACHE["nc"]


def _common_inputs(inputs):
    common = {}
    for nm in ("q", "k", "v", "p", "f1", "f2"):
        w_eff, b_eff = _eff_params(inputs, nm)
        common[f"w_{nm}"] = _wlayout(w_eff)
        if nm in ("k", "v"):
            common[f"b_{nm}"] = np.ascontiguousarray(
                np.broadcast_to(b_eff[None, :], (P, C))).astype(np.float32)
        else:
            common[f"b_{nm}"] = _blayout_T(b_eff)
    return common


def _shard_x(x, i):
    """core i's x shard -> [P, KC, NT] bf16 transposed layout."""
    xc = np.asarray(x[i * BPC:(i + 1) * BPC], dtype=np.float32)  # (BPC, N, C)
    xt = xc.reshape(NT, C).T                                     # [C, NT]
    xt = xt.reshape(KC, P, NT).transpose(1, 0, 2)                # [P, KC, NT]
    return np.ascontiguousarray(xt).astype(NPBF)


def _unshard_out(res):
    """[P, KC, NT] f32 -> (BPC, N, C) f32."""
    yt = res.transpose(1, 0, 2).reshape(C, NT)                   # [C, NT]
    return np.ascontiguousarray(yt.T.reshape(BPC, N, C))


def run(inputs, trace=False, **kwargs):
    nc = _get_nc()
    common = _common_inputs(inputs)
    in_maps = [dict(common, x_d=_shard_x(inputs["x"], i)) for i in range(NCORES)]
    res = run_bass_kernel_spmd(nc, in_maps, core_ids=list(range(NCORES)),
                               trace=trace, **kwargs)
    y = np.empty((B, N, C), dtype=np.float32)
    for i in range(NCORES):
        y[i * BPC:(i + 1) * BPC] = _unshard_out(res.results[i]["out"])
    return y, res


def kernel(**inputs):
    y, _ = run(inputs, trace=False)
    return y


# revision 17
# speedup vs baseline: 1.0305x; 1.0167x over previous
"""Trainium2 Bass kernel for nn_Block_85633057947963 (dense transformer block).

Strategy:
  - Data-parallel over batch: 16 batches -> 8 cores x 2 batches (2048 tokens/core).
  - BatchNorm (fixed running stats) folded into each linear on the host:
        stage(x) = relu(x @ w_eff + b_eff),
        w_eff = w.T * s, b_eff = (b - m) * s + be, s = g / sqrt(v + eps).
  - Linear attention reassociated: (q k^T * 0.125) v == q (0.125 * k^T v),
    turning two [N,N] matmuls into two tiny [64,64]-contraction matmuls.
  - Activations kept in transposed layout [C, tokens] (channels on SBUF
    partitions) so every linear is matmul(lhsT=w_eff, rhs=actT) with zero
    on-device transposes, and per-channel bias+ReLU ride the ACT engine's
    free affine. k and v are additionally produced in natural layout
    [tokens, C] for the k^T v contraction.
  - Matmul operands in bf16 (1 cyc/row on the PE vs 2 for fp32), fp32 PSUM
    accumulation, fp32 epilogues; the x1 residual is kept in fp32.
"""

import numpy as np
import ml_dtypes

import concourse.bass as bass
import concourse.tile as tile
from concourse import bacc, mybir
from concourse.bass_utils import run_bass_kernel_spmd

# Problem shapes (hardcoded per spec)
B, N, C, H, HID = 16, 1024, 512, 8, 2048
BN_EPS = 1e-5
NCORES = 8
BPC = B // NCORES          # batches per core = 2
NT = BPC * N               # tokens per core = 2048
P = 128
KC = C // P                # 4 chunks of input channels
HC = HID // P              # 16 chunks of hidden channels
DH = C // H                # 64 = head dim
TCH = 512                  # token chunk (matmul free dim)
NTC = NT // TCH            # 4 token chunks per core
TPB = N // P               # 8 chunks of 128 tokens per batch

F32 = mybir.dt.float32
BF16 = mybir.dt.bfloat16
NPBF = ml_dtypes.bfloat16
RELU = mybir.ActivationFunctionType.Relu
ADD = mybir.AluOpType.add


def _build_nc(debug=False):
    nc = bacc.Bacc("TRN2", target_bir_lowering=False, debug=debug,
                   num_devices=NCORES)

    x_d = nc.dram_tensor("x_d", [P, KC, NT], BF16, kind="ExternalInput").ap()
    w_in = {}
    for nm, kc, cout in (("q", KC, C), ("k", KC, C), ("v", KC, C),
                         ("p", KC, C), ("f1", KC, HID), ("f2", HC, C)):
        w_in[nm] = nc.dram_tensor(f"w_{nm}", [P, kc, cout], BF16,
                                  kind="ExternalInput").ap()
    b_in = {}
    for nm, nch in (("q", KC), ("p", KC), ("f1", HC), ("f2", KC)):
        b_in[nm] = nc.dram_tensor(f"b_{nm}", [P, nch], F32,
                                  kind="ExternalInput").ap()
    for nm in ("k", "v"):
        b_in[nm] = nc.dram_tensor(f"b_{nm}", [P, C], F32,
                                  kind="ExternalInput").ap()
    out_d = nc.dram_tensor("out", [P, KC, NT], F32, kind="ExternalOutput").ap()

    with tile.TileContext(nc) as tc:
        with (
            tc.tile_pool(name="wpool", bufs=1) as wpool,
            tc.tile_pool(name="bpool", bufs=1) as bpool,
            tc.tile_pool(name="actD", bufs=2) as actD,       # x / qT / oT
            tc.tile_pool(name="kv", bufs=2) as kvpool,       # k_nat, v_nat
            tc.tile_pool(name="x1f", bufs=1) as x1pool,      # x1 fp32
            tc.tile_pool(name="hp", bufs=1) as hpool,        # FFN hidden
            tc.tile_pool(name="tmpf", bufs=4) as tmpf,       # fp32 [P,TCH]
            tc.tile_pool(name="tmpd", bufs=3) as tmpd,       # bf16 [P,TCH]
            tc.tile_pool(name="castd", bufs=5) as castd,     # x1 bf16 slices
            tc.tile_pool(name="xres", bufs=3) as xres,       # x stream for res1
            tc.tile_pool(name="spool", bufs=9) as spool,     # S head-pair tiles
            tc.tile_pool(name="psA", bufs=7, space="PSUM") as psA,
        ):
            # ---- resident weights & biases.  DMA order matters: the input
            # stream is ~8MB at per-core HBM bandwidth (~22us serial), so
            # emit (biases, w_q, x-chunks) first and the FFN weights last,
            # letting the first q matmuls start after ~1MB has landed.
            wt = {}
            bt = {}

            def load_w(nm, kc, cout, eng):
                t = wpool.tile([P, kc, cout], BF16, tag=f"w_{nm}")
                eng.dma_start(t[:], w_in[nm])
                wt[nm] = t

            # Early-needed small weights + biases issue from the (otherwise
            # idle) GpSimd queue, in parallel with Sync issuing the x chunks.
            # The big FFN weights stay on Sync BEHIND the x chunks so their
            # transfers don't steal HBM bandwidth from the critical prefix.
            load_w("q", KC, C, nc.gpsimd)
            xT = actD.tile([P, KC, NT], BF16, tag="big")
            for t in range(NTC):
                nc.sync.dma_start(xT[:, :, t * TCH:(t + 1) * TCH],
                                  x_d[:, :, t * TCH:(t + 1) * TCH])
            load_w("k", KC, C, nc.gpsimd)
            load_w("v", KC, C, nc.gpsimd)
            for nm, nch in (("q", KC), ("p", KC), ("f1", HC), ("f2", KC)):
                t = bpool.tile([P, nch], F32, tag=f"b_{nm}")
                nc.gpsimd.dma_start(t[:], b_in[nm])
                bt[nm] = t
            for nm in ("k", "v"):
                t = bpool.tile([P, C], F32, tag=f"b_{nm}")
                nc.gpsimd.dma_start(t[:], b_in[nm])
                bt[nm] = t
            load_w("p", KC, C, nc.sync)
            load_w("f1", KC, HID, nc.sync)
            load_w("f2", HC, C, nc.sync)

            # explicit zero bias (avoids a const-tensor preamble load)
            zbias = bpool.tile([P, 1], F32, tag="zb")
            nc.vector.memset(zbias[:], 0.0)

            # pre-warm the PE HAM clock gate with junk matmuls while the
            # input DMAs stream in (results discarded)
            warm_w = bpool.tile([P, TCH], BF16, tag="warm")
            nc.vector.memset(warm_w[:], 0.0)
            ps_warm = psA.tile([P, TCH], F32, tag="warm", bufs=1)
            for _ in range(11):
                nc.tensor.matmul(ps_warm[:], warm_w[:, :P], warm_w[:],
                                 start=True, stop=True)

            # ---- phase 1: projections
            qT = actD.tile([P, KC, NT], BF16, tag="big")
            kN = kvpool.tile([P, BPC * TPB, C], BF16, tag="kv")
            vN = kvpool.tile([P, BPC * TPB, C], BF16, tag="kv")

            for t in range(NTC):          # q: transposed output; token-outer
                for ch in range(KC):      # so MMs start once x chunk 0 lands
                    ps = psA.tile([P, TCH], F32, tag="mm")
                    for ks in range(KC):
                        nc.tensor.matmul(ps[:], wt["q"][:, ks, ch * P:(ch + 1) * P],
                                         xT[:, ks, t * TCH:(t + 1) * TCH],
                                         start=(ks == 0), stop=(ks == KC - 1))
                    nc.scalar.activation(qT[:, ch, t * TCH:(t + 1) * TCH], ps[:],
                                         RELU, bias=bt["q"][:, ch:ch + 1])

            for nm, dst in (("k", kN), ("v", vN)):   # k, v: natural output
                for tch in range(BPC * TPB):
                    ps = psA.tile([P, C], F32, tag="mm")
                    for ks in range(KC):
                        nc.tensor.matmul(ps[:], xT[:, ks, tch * P:(tch + 1) * P],
                                         wt[nm][:, ks, :],
                                         start=(ks == 0), stop=(ks == KC - 1))
                    tmp = tmpd.tile([P, C], BF16, tag="kvtmp")
                    nc.vector.tensor_tensor(tmp[:], ps[:], bt[nm][:], ADD)
                    nc.scalar.activation(dst[:, tch, :], tmp[:], RELU, bias=zbias[:])

            # ---- phase 2: attention (associative).  The two heads of each
            # 128-channel pair are packed into one PSUM tile at partition
            # bases 0/64 (tile_position auto-derives from the AP bases), so
            # the two M=64 matmul streams run on disjoint PE column groups
            # and each epilogue is a single [128, .] ACT op.
            oT = actD.tile([P, KC, NT], BF16, tag="big")
            Sps = {}
            for b in range(BPC):           # pass 1: all k^T v chains
                for hp0 in (0, 2):         # two chains interleaved
                    pss = {}
                    for hp in (hp0, hp0 + 1):
                        Sps[b, hp] = spool.tile([P, DH], BF16, tag="S",
                                                name=f"Sp_{b}_{hp}")
                        ps_full = psA.tile([P, TCH], F32, tag="mm",
                                           name=f"ps_s{b}_{hp}")
                        pss[hp] = ps_full[:, :DH]
                    for j in range(TPB):
                        tch = b * TPB + j
                        for hp in (hp0, hp0 + 1):
                            for sub in range(2):
                                h = hp * 2 + sub
                                nc.tensor.matmul(
                                    pss[hp][sub * DH:(sub + 1) * DH, :],
                                    kN[:, tch, h * DH:(h + 1) * DH],
                                    vN[:, tch, h * DH:(h + 1) * DH],
                                    start=(j == 0), stop=(j == TPB - 1),
                                    skip_group_check=True)
                    for hp in (hp0, hp0 + 1):
                        nc.scalar.mul(Sps[b, hp][:], pss[hp][:], 0.125)
            # pass 2 (o = q S) interleaved per token chunk with phase 3
            # (p projection + residual 1) so the PE never waits on epilogues
            x1 = x1pool.tile([P, KC, NT], F32, tag="x1")
            for tg in range(NTC):          # global token chunk
                b, t = divmod(tg, N // TCH)
                tok0 = b * N + t * TCH
                for hp in range(KC):
                    Sp = Sps[b, hp]
                    ps_o = psA.tile([P, TCH], F32, tag="mm")
                    for sub in range(2):
                        nc.tensor.matmul(ps_o[sub * DH:(sub + 1) * DH, :],
                                         Sp[sub * DH:(sub + 1) * DH, :],
                                         qT[sub * DH:(sub + 1) * DH, hp,
                                            tok0:tok0 + TCH],
                                         start=True, stop=True,
                                         skip_group_check=True)
                    nc.scalar.activation(oT[:, hp, tok0:tok0 + TCH],
                                         ps_o[:], RELU, bias=zbias[:])
                if tg == 0:
                    continue  # p for chunk tg-1 emitted below once o ready
                for ch in range(KC):
                    tp = tg - 1
                    ps = psA.tile([P, TCH], F32, tag="mm")
                    for ks in range(KC):
                        nc.tensor.matmul(ps[:], wt["p"][:, ks, ch * P:(ch + 1) * P],
                                         oT[:, ks, tp * TCH:(tp + 1) * TCH],
                                         start=(ks == 0), stop=(ks == KC - 1))
                    tf = tmpf.tile([P, TCH], F32, tag="tf")
                    nc.scalar.activation(tf[:], ps[:], RELU, bias=bt["p"][:, ch:ch + 1])
                    xr = xres.tile([P, TCH], BF16, tag="xr")
                    nc.sync.dma_start(xr[:], x_d[:, ch, tp * TCH:(tp + 1) * TCH])
                    nc.vector.tensor_tensor(x1[:, ch, tp * TCH:(tp + 1) * TCH],
                                            tf[:], xr[:], ADD)
            for ch in range(KC):           # p for the last token chunk
                tp = NTC - 1
                ps = psA.tile([P, TCH], F32, tag="mm")
                for ks in range(KC):
                    nc.tensor.matmul(ps[:], wt["p"][:, ks, ch * P:(ch + 1) * P],
                                     oT[:, ks, tp * TCH:(tp + 1) * TCH],
                                     start=(ks == 0), stop=(ks == KC - 1))
                tf = tmpf.tile([P, TCH], F32, tag="tf")
                nc.scalar.activation(tf[:], ps[:], RELU, bias=bt["p"][:, ch:ch + 1])
                xr = xres.tile([P, TCH], BF16, tag="xr")
                nc.sync.dma_start(xr[:], x_d[:, ch, tp * TCH:(tp + 1) * TCH])
                nc.vector.tensor_tensor(x1[:, ch, tp * TCH:(tp + 1) * TCH],
                                        tf[:], xr[:], ADD)

            # ---- phase 4: FFN + residual 2, per token chunk
            for t in range(NTC):
                x1d = []
                for ks in range(KC):
                    cd = castd.tile([P, TCH], BF16, tag="x1d")
                    nc.vector.tensor_copy(cd[:], x1[:, ks, t * TCH:(t + 1) * TCH])
                    x1d.append(cd)
                hT = hpool.tile([P, HC, TCH], BF16, tag="h")
                for hch in range(HC):
                    ps = psA.tile([P, TCH], F32, tag="mm")
                    for ks in range(KC):
                        nc.tensor.matmul(ps[:], wt["f1"][:, ks, hch * P:(hch + 1) * P],
                                         x1d[ks][:],
                                         start=(ks == 0), stop=(ks == KC - 1))
                    # relu(psum + bias): alternate DVE/ACT to balance engines
                    if hch % 2 == 0:
                        nc.vector.tensor_scalar(hT[:, hch, :], ps[:],
                                                bt["f1"][:, hch:hch + 1], 0.0,
                                                ADD, mybir.AluOpType.max)
                    else:
                        nc.scalar.activation(hT[:, hch, :], ps[:], RELU,
                                             bias=bt["f1"][:, hch:hch + 1])
                for ch in range(KC):
                    ps = psA.tile([P, TCH], F32, tag="mm")
                    for ks in range(HC):
                        nc.tensor.matmul(ps[:], wt["f2"][:, ks, ch * P:(ch + 1) * P],
                                         hT[:, ks, :],
                                         start=(ks == 0), stop=(ks == HC - 1))
                    tf = tmpf.tile([P, TCH], F32, tag="tf")
                    nc.scalar.activation(tf[:], ps[:], RELU, bias=bt["f2"][:, ch:ch + 1])
                    of = tmpf.tile([P, TCH], F32, tag="tf")
                    nc.vector.tensor_tensor(of[:], tf[:],
                                            x1[:, ch, t * TCH:(t + 1) * TCH], ADD)
                    nc.sync.dma_start(out_d[:, ch, t * TCH:(t + 1) * TCH], of[:])

    nc.compile()
    return nc


def _eff_params(inputs, pref):
    w = inputs[pref + "_w"].astype(np.float32)
    b = inputs[pref + "_b"].astype(np.float32)
    g = inputs[pref + "_g"].astype(np.float32)
    be = inputs[pref + "_be"].astype(np.float32)
    m = inputs[pref + "_m"].astype(np.float32)
    v = inputs[pref + "_v"].astype(np.float32)
    s = g / np.sqrt(v + BN_EPS)
    w_eff = (w.T * s).astype(np.float32)          # [C_in, C_out]
    b_eff = ((b - m) * s + be).astype(np.float32)
    return w_eff, b_eff


def _wlayout(w_eff):
    """[C_in, C_out] -> [P, C_in//P, C_out] with channel-in striped on partitions."""
    cin, cout = w_eff.shape
    return np.ascontiguousarray(
        w_eff.reshape(cin // P, P, cout).transpose(1, 0, 2)).astype(NPBF)


def _blayout_T(b_eff):
    """per-channel bias -> [P, nch] (channel chunks on free dim)."""
    n = b_eff.shape[0]
    return np.ascontiguousarray(b_eff.reshape(n // P, P).T).astype(np.float32)


_CACHE = {}


def _get_nc():
    if "nc" not in _CACHE:
        _CACHE["nc"] = _build_nc(debug=False)
    return _CACHE["nc"]


def _common_inputs(inputs):
    common = {}
    for nm in ("q", "k", "v", "p", "f1", "f2"):
        w_eff, b_eff = _eff_params(inputs, nm)
        common[f"w_{nm}"] = _wlayout(w_eff)
        if nm in ("k", "v"):
            common[f"b_{nm}"] = np.ascontiguousarray(
                np.broadcast_to(b_eff[None, :], (P, C))).astype(np.float32)
        else:
            common[f"b_{nm}"] = _blayout_T(b_eff)
    return common


def _shard_x(x, i):
    """core i's x shard -> [P, KC, NT] bf16 transposed layout."""
    xc = np.asarray(x[i * BPC:(i + 1) * BPC], dtype=np.float32)  # (BPC, N, C)
    xt = xc.reshape(NT, C).T                                     # [C, NT]
    xt = xt.reshape(KC, P, NT).transpose(1, 0, 2)                # [P, KC, NT]
    return np.ascontiguousarray(xt).astype(NPBF)


def _unshard_out(res):
    """[P, KC, NT] f32 -> (BPC, N, C) f32."""
    yt = res.transpose(1, 0, 2).reshape(C, NT)                   # [C, NT]
    return np.ascontiguousarray(yt.T.reshape(BPC, N, C))


def run(inputs, trace=False, **kwargs):
    nc = _get_nc()
    common = _common_inputs(inputs)
    in_maps = [dict(common, x_d=_shard_x(inputs["x"], i)) for i in range(NCORES)]
    res = run_bass_kernel_spmd(nc, in_maps, core_ids=list(range(NCORES)),
                               trace=trace, **kwargs)
    y = np.empty((B, N, C), dtype=np.float32)
    for i in range(NCORES):
        y[i * BPC:(i + 1) * BPC] = _unshard_out(res.results[i]["out"])
    return y, res


def kernel(**inputs):
    y, _ = run(inputs, trace=False)
    return y
